# revision 20
# baseline (speedup 1.0000x reference)
"""Multi-head attention (B=2, N=4096, C=512, H=8, D=64) on 8 TRN2 NeuronCores.

Sharding: data-parallel over batch (2 groups of 4 cores) x tensor-parallel over
heads (2 heads/core). Per core: qkv projection, attention for its 2 heads, and
a partial output projection; the host sums the 4 per-batch partials, transposes,
adds bias.

v2 engine layout (from v1 profiling: ScalarE exp was 97% busy and paced the
kernel; PE ~85%):
- exp is split across TWO engines: ScalarE runs real exp (table ACT) on 9/16 of
  key-chunks; the DVE computes a Schraudolph-style exp on the other 7/16 with a
  single tensor_scalar: i16 = rint(S' + BOFS) bit-cast as bf16, where
  S' = logits * 128 * log2(e) (the scale is folded into the Q weights on host).
  Softmax renormalization cancels the approximation's mean scale; measured
  end-to-end rel err ~8.6e-3 vs the 2e-2 budget.
- QK uses row-tiled 64-contraction matmul pairs: head0 on PE rows 0-63, head1
  on rows 64-127 run CONCURRENTLY (distinct row groups), halving QK time vs
  the zero-padded 128-contraction variant.
- Softmax denominator rides the PV matmul as a ones-column appended to V
  (lhsT = [V_h | 1], M=65); the denominator row is moved to partition 0 by a
  small DMA, inverted on DVE, partition-broadcast and multiplied on GpSimd.
- All inputs arrive bf16 from the host (no on-chip staging casts); the partial
  output is written bf16 (host accumulates in f32).
"""
import os
import sys

for _p in ("/opt/trn_rl_repo", "/root/.axon_site/_ro/trn_rl_repo"):
    if os.path.isdir(_p) and _p not in sys.path:
        sys.path.append(_p)

import numpy as np
from contextlib import ExitStack

import concourse.bass as bass
import concourse.mybir as mybir
import concourse.tile as tile
from concourse import bacc
from concourse.bass_utils import run_bass_kernel_spmd

F32 = mybir.dt.float32
BF16 = mybir.dt.bfloat16
I16 = mybir.dt.int16
EXP = mybir.ActivationFunctionType.Exp
ADD = mybir.AluOpType.add

DIM, N, HD = 512, 4096, 64
CC = DIM // 128  # 4 c-chunks of the model dim
NB = N // 512    # 8 n-blocks of 512 queries
MB = N // 128    # 32 m-chunks of 128 keys

# exp(L) = 2^(L*log2e) ; S' = L * 128 * log2e is folded into Wq on the host.
CFAC = float((HD ** -0.5) * 128.0 / np.log(2.0))   # host-side Wq prescale
SCALE_SC = float(np.log(2.0) / 128.0)              # ScalarE: exp(S' * this) = e^L
BOFS = float(127 * 128 - 3.5)                      # DVE: bf16 bits = rint(S'+BOFS)
# m % 16 in this set -> DVE schraudolph exp (7/16 of chunks)
DVE_M = frozenset((1, 3, 5, 8, 10, 12, 14))

BF16_NP = mybir.dt.np(BF16)


def build_nc(n=N):
    nb_cnt, mb_cnt = n // 512, n // 128
    nc = bacc.Bacc("TRN2", target_bir_lowering=False)
    xT = nc.declare_dram_parameter("xT", [DIM, n], BF16, isOutput=False)
    wqkvT = nc.declare_dram_parameter("wqkvT", [DIM, 384], BF16, isOutput=False)
    wpT = nc.declare_dram_parameter("wpT", [128, DIM], BF16, isOutput=False)
    out = nc.declare_dram_parameter("out", [DIM, n], BF16, isOutput=True)

    with ExitStack() as ctx:
        tc = ctx.enter_context(tile.TileContext(nc))
        big = ctx.enter_context(tc.tile_pool(name="big", bufs=1))
        esp = ctx.enter_context(tc.tile_pool(name="esp", bufs=5))
        nrm = ctx.enter_context(tc.tile_pool(name="nrm", bufs=2))
        ysp = ctx.enter_context(tc.tile_pool(name="ysp", bufs=3))
        ps_T = ctx.enter_context(tc.tile_pool(name="psT", bufs=3, space="PSUM"))
        ps_O = ctx.enter_context(tc.tile_pool(name="psO", bufs=2, space="PSUM"))

        # ---- persistent tiles + weight loads (all bf16, direct DMA) ----
        wq = []
        for cc in range(CC):
            t = big.tile([128, 384], BF16, tag=f"wq{cc}", name=f"wq{cc}")
            nc.sync.dma_start(out=t[:], in_=wqkvT[cc * 128:(cc + 1) * 128, :])
            wq.append(t)
        wpb = big.tile([128, DIM], BF16, tag="wpb", name="wpb")
        nc.sync.dma_start(out=wpb[:], in_=wpT[:, :])
        xtb = [big.tile([128, n], BF16, tag=f"xtb{cc}", name=f"xtb{cc}")
               for cc in range(CC)]
        qt = big.tile([128, n], BF16, tag="qt", name="qt")
        ktB = big.tile([128, n], BF16, tag="ktB", name="ktB")
        v2 = big.tile([128, 130 * mb_cnt], BF16, tag="v2", name="v2")
        nc.vector.memset(v2[:], 1.0)
        atB = big.tile([128, n], BF16, tag="atB", name="atB")

        # ---- prep: per n-block compute Q^T, K^T, V (x DMAs issued upfront) ----
        for nb in range(nb_cnt):
            ns = slice(nb * 512, (nb + 1) * 512)
            for cc in range(CC):
                nc.sync.dma_start(out=xtb[cc][:, ns],
                                  in_=xT[cc * 128:(cc + 1) * 128, ns])

        def emit_prep(nb):
            ns = slice(nb * 512, (nb + 1) * 512)
            psqk = ps_T.tile([128, 1024], F32, tag="T", name="psqk")
            for blk in range(2):  # 0: Q rows (prescaled), 1: K rows
                for cc in range(CC):
                    nc.tensor.matmul(
                        psqk[:, blk * 512:(blk + 1) * 512],
                        lhsT=wq[cc][:, blk * 128:(blk + 1) * 128],
                        rhs=xtb[cc][:, ns],
                        start=(cc == 0),
                        stop=(cc == CC - 1),
                    )
            nc.scalar.copy(out=qt[:, ns], in_=psqk[:, 0:512])
            nc.scalar.copy(out=ktB[:, ns], in_=psqk[:, 512:1024])
            # V for the 4 key-chunks of this n-block: [128 keys, 128 vchan] each
            psv = ps_T.tile([128, 1024], F32, tag="T", name="psv")
            for j in range(4):
                mb = nb * 4 + j
                for cc in range(CC):
                    nc.tensor.matmul(
                        psv[:, j * 128:(j + 1) * 128],
                        lhsT=xtb[cc][:, mb * 128:(mb + 1) * 128],
                        rhs=wq[cc][:, 256:384],
                        start=(cc == 0),
                        stop=(cc == CC - 1),
                    )
            # scatter into v2 layout [1 | V_h0(64) | 1 | V_h1(64)] per chunk:
            # the leading ones column makes the PV denominator land on PSUM
            # partition 0, where partition_broadcast can read it directly
            src = psv[:, 0:512].rearrange("p (j c) -> p j c", c=128)
            dst = v2[:, nb * 4 * 130:(nb + 1) * 4 * 130].rearrange(
                "p (j c) -> p j c", c=130)
            for h in range(2):
                nc.vector.tensor_copy(
                    out=dst[:, :, h * 65 + 1:h * 65 + 65],
                    in_=src[:, :, h * 64:(h + 1) * 64],
                )

        emit_prep(0)
        if nb_cnt > 1:
            emit_prep(1)

        # ---- attention ----
        # po rows: 0 = denominator (leading ones column), 1:65 = y_u
        def emit_norms(nb, po0, po1):
            ns = slice(nb * 512, (nb + 1) * 512)
            yvs, recs = [], []
            for po in (po0, po1):
                yu = nrm.tile([65, 512], F32, tag="yu", name="yu")
                nc.vector.tensor_copy(out=yu[:], in_=po[0:65, :])
                rc1 = nrm.tile([1, 512], F32, tag="rc1", name="rc1")
                nc.vector.reciprocal_approx_fast(out=rc1[:], in_=yu[0:1, :])
                # partition shift 1:65 -> 0:64 rides the DMA engine, hidden
                # behind the reciprocal + broadcast chain
                yv = nrm.tile([64, 512], F32, tag="yv", name="yv")
                nc.sync.dma_start(out=yv[:], in_=yu[1:65, :])
                yvs.append(yv)
                recs.append(rc1)
            brs = []
            for rc1 in recs:
                rec = nrm.tile([64, 512], F32, tag="rec", name="rec")
                nc.gpsimd.partition_broadcast(rec[:], rc1[0:1, :])
                brs.append(rec)
            nc.vector.tensor_mul(out=atB[0:64, ns], in0=yvs[0][:],
                                 in1=brs[0][:])
            a1 = nrm.tile([64, 512], BF16, tag="a1", name="a1")
            nc.vector.tensor_mul(out=a1[:], in0=yvs[1][:], in1=brs[1][:])
            nc.sync.dma_start(out=atB[64:128, ns], in_=a1[:])

        def emit_proj_ob(nb, ob):
            ns = slice(nb * 512, (nb + 1) * 512)
            pp = ps_T.tile([128, 1024], F32, tag="T", name="pp")[:, 0:512]
            nc.tensor.matmul(
                pp,
                lhsT=wpb[:, ob * 128:(ob + 1) * 128],
                rhs=atB[:, ns],
                start=True,
                stop=True,
            )
            ys = ysp.tile([128, 512], BF16, tag="ys", name="ys")
            nc.scalar.copy(out=ys[:], in_=pp)
            nc.sync.dma_start(out=out[ob * 128:(ob + 1) * 128, ns], in_=ys[:])

        def emit_pv(m, es, po0, po1):
            for h, po in ((0, po0), (1, po1)):
                nc.tensor.matmul(
                    po[0:65, :],
                    lhsT=v2[:, m * 130 + 65 * h:m * 130 + 65 * h + 65],
                    rhs=es[:, h * 512:(h + 1) * 512],
                    start=(m == 0),
                    stop=(m == mb_cnt - 1),
                )

        # PV lags QK/exp by 2 chunks so exp latency stays off the critical
        # path; prep for n-blocks 1.. is interleaved into nb=0's chunk loop.
        pend = None  # (nb, po0, po1, es[m-2], es[m-1])
        for nb in range(nb_cnt):
            ns = slice(nb * 512, (nb + 1) * 512)
            po0 = ps_O.tile([128, 512], F32, tag="po", name="po0")
            po1 = ps_O.tile([128, 512], F32, tag="po", name="po1")
            es_hist = {}
            for m in range(mb_cnt):
                ks = slice(m * 128, (m + 1) * 128)
                T = ps_T.tile([128, 1024], F32, tag="T", name="T")
                # row-tiled QK pair: head0 on PE rows 0-63, head1 on 64-127
                nc.tensor.matmul(T[:, 0:512], lhsT=ktB[0:64, ks],
                                 rhs=qt[0:64, ns], start=True, stop=True)
                nc.tensor.matmul(T[:, 512:1024], lhsT=ktB[64:128, ks],
                                 rhs=qt[64:128, ns], start=True, stop=True)
                es = esp.tile([128, 1024], BF16, tag="es", name="es")
                if (m % 16) in DVE_M:
                    nc.vector.tensor_scalar(
                        out=es[:].bitcast(I16), in0=T[:],
                        scalar1=BOFS, scalar2=None, op0=ADD)
                else:
                    nc.scalar.activation(out=es[:], in_=T[:], func=EXP,
                                         scale=SCALE_SC)
                es_hist[m] = es
                if pend is not None:
                    pnb, ppo0, ppo1, pes2, pes1 = pend
                    if m == 0:
                        emit_pv(mb_cnt - 2, pes2, ppo0, ppo1)
                    elif m == 1:
                        emit_pv(mb_cnt - 1, pes1, ppo0, ppo1)
                        emit_norms(pnb, ppo0, ppo1)
                        pend = None
                if nb == 0 and m % 4 == 0 and m <= 20 \
                        and (m // 4 + 2) < nb_cnt:
                    emit_prep(m // 4 + 2)
                if 10 <= m < 14 and nb > 0:
                    emit_proj_ob(nb - 1, m - 10)
                if m >= 2:
                    emit_pv(m - 2, es_hist.pop(m - 2), po0, po1)
            pend = (nb, po0, po1, es_hist.pop(mb_cnt - 2),
                    es_hist.pop(mb_cnt - 1))

        pnb, ppo0, ppo1, pes2, pes1 = pend
        emit_pv(mb_cnt - 2, pes2, ppo0, ppo1)
        emit_pv(mb_cnt - 1, pes1, ppo0, ppo1)
        emit_norms(pnb, ppo0, ppo1)
        for ob in range(4):
            emit_proj_ob(nb_cnt - 1, ob)

    nc.compile()
    return nc


_NC_CACHE = None
LAST_EXEC_NS = None


def kernel(x, w_qkv, w_proj, b_proj):
    global _NC_CACHE, LAST_EXEC_NS
    x = np.asarray(x, dtype=np.float32)
    w_qkv = np.asarray(w_qkv, dtype=np.float32)
    w_proj = np.asarray(w_proj, dtype=np.float32)
    b_proj = np.asarray(b_proj, dtype=np.float32)
    B = x.shape[0]

    if _NC_CACHE is None:
        _NC_CACHE = build_nc()
    nc = _NC_CACHE

    xTs = [np.ascontiguousarray(x[b].T.astype(BF16_NP)) for b in range(B)]
    in_maps = []
    for c in range(8):
        b, hp = c // 4, c % 4
        qr = w_qkv[2 * hp * 64:2 * hp * 64 + 128] * np.float32(CFAC)
        kr = w_qkv[512 + 2 * hp * 64:512 + 2 * hp * 64 + 128]
        vr = w_qkv[1024 + 2 * hp * 64:1024 + 2 * hp * 64 + 128]
        wqkvT = np.ascontiguousarray(
            np.concatenate([qr, kr, vr], 0).T.astype(BF16_NP))
        wpT = np.ascontiguousarray(
            w_proj[:, hp * 128:(hp + 1) * 128].T.astype(BF16_NP))
        in_maps.append({"xT": xTs[b], "wqkvT": wqkvT, "wpT": wpT})

    res = run_bass_kernel_spmd(
        nc,
        in_maps,
        core_ids=list(range(8)),
        trace=bool(int(os.environ.get("ATTN_TRACE", "0"))),
    )
    LAST_EXEC_NS = res.exec_time_ns

    out = np.zeros((B, N, DIM), np.float32)
    for b in range(B):
        acc = res.results[4 * b]["out"].astype(np.float32)
        for c in range(4 * b + 1, 4 * b + 4):
            acc += res.results[c]["out"].astype(np.float32)
        out[b] = acc.T + b_proj
    return out


# revision 23
# speedup vs baseline: 1.1158x; 1.1158x over previous
"""Multi-head attention (B=2, N=4096, C=512, H=8, D=64) on 8 TRN2 NeuronCores.

Sharding: data-parallel over batch (2 groups of 4 cores) x tensor-parallel over
heads (2 heads/core). Per core: qkv projection, attention for its 2 heads, and
a partial output projection; the host sums the 4 per-batch partials, transposes,
adds bias.

v2 engine layout (from v1 profiling: ScalarE exp was 97% busy and paced the
kernel; PE ~85%):
- exp is split across TWO engines: ScalarE runs real exp (table ACT) on 9/16 of
  key-chunks; the DVE computes a Schraudolph-style exp on the other 7/16 with a
  single tensor_scalar: i16 = rint(S' + BOFS) bit-cast as bf16, where
  S' = logits * 128 * log2(e) (the scale is folded into the Q weights on host).
  Softmax renormalization cancels the approximation's mean scale; measured
  end-to-end rel err ~8.6e-3 vs the 2e-2 budget.
- QK uses row-tiled 64-contraction matmul pairs: head0 on PE rows 0-63, head1
  on rows 64-127 run CONCURRENTLY (distinct row groups), halving QK time vs
  the zero-padded 128-contraction variant.
- Softmax denominator rides the PV matmul as a ones-column appended to V
  (lhsT = [V_h | 1], M=65); the denominator row is moved to partition 0 by a
  small DMA, inverted on DVE, partition-broadcast and multiplied on GpSimd.
- All inputs arrive bf16 from the host (no on-chip staging casts); the partial
  output is written bf16 (host accumulates in f32).
"""
import os
import sys

for _p in ("/opt/trn_rl_repo", "/root/.axon_site/_ro/trn_rl_repo"):
    if os.path.isdir(_p) and _p not in sys.path:
        sys.path.append(_p)

import numpy as np
from contextlib import ExitStack

import concourse.bass as bass
import concourse.mybir as mybir
import concourse.tile as tile
from concourse import bacc
from concourse.bass_utils import run_bass_kernel_spmd

F32 = mybir.dt.float32
BF16 = mybir.dt.bfloat16
I16 = mybir.dt.int16
EXP = mybir.ActivationFunctionType.Exp
ADD = mybir.AluOpType.add

DIM, N, HD = 512, 4096, 64
CC = DIM // 128  # 4 c-chunks of the model dim
NB = N // 512    # 8 n-blocks of 512 queries
MB = N // 128    # 32 m-chunks of 128 keys

# exp(L) = 2^(L*log2e) ; S' = L * 128 * log2e is folded into Wq on the host.
CFAC = float((HD ** -0.5) * 128.0 / np.log(2.0))   # host-side Wq prescale
SCALE_SC = float(np.log(2.0) / 128.0)              # ScalarE: exp(S' * this) = e^L
BOFS = float(127 * 128 - 3.5)                      # DVE: bf16 bits = rint(S'+BOFS)
# m % 16 in this set -> DVE schraudolph exp (7/16 of chunks)
DVE_M = frozenset((1, 3, 5, 8, 10, 12, 14))

BF16_NP = mybir.dt.np(BF16)


def build_nc(n=N):
    nb_cnt, mb_cnt = n // 512, n // 128
    nc = bacc.Bacc("TRN2", target_bir_lowering=False)
    xT = nc.declare_dram_parameter("xT", [DIM, n], BF16, isOutput=False)
    wqkvT = nc.declare_dram_parameter("wqkvT", [DIM, 384], BF16, isOutput=False)
    wpT = nc.declare_dram_parameter("wpT", [128, DIM], BF16, isOutput=False)
    out = nc.declare_dram_parameter("out", [DIM, n], BF16, isOutput=True)

    with ExitStack() as ctx:
        tc = ctx.enter_context(tile.TileContext(nc))
        big = ctx.enter_context(tc.tile_pool(name="big", bufs=1))
        esp = ctx.enter_context(tc.tile_pool(name="esp", bufs=7))
        nrm = ctx.enter_context(tc.tile_pool(name="nrm", bufs=2))
        ysp = ctx.enter_context(tc.tile_pool(name="ysp", bufs=3))
        ps_T = ctx.enter_context(tc.tile_pool(name="psT", bufs=3, space="PSUM"))
        ps_O = ctx.enter_context(tc.tile_pool(name="psO", bufs=2, space="PSUM"))

        # ---- persistent tiles + weight loads (all bf16, direct DMA) ----
        wq = []
        for cc in range(CC):
            t = big.tile([128, 384], BF16, tag=f"wq{cc}", name=f"wq{cc}")
            nc.sync.dma_start(out=t[:], in_=wqkvT[cc * 128:(cc + 1) * 128, :])
            wq.append(t)
        wpb = big.tile([128, DIM], BF16, tag="wpb", name="wpb")
        nc.sync.dma_start(out=wpb[:], in_=wpT[:, :])
        xtb = [big.tile([128, n], BF16, tag=f"xtb{cc}", name=f"xtb{cc}")
               for cc in range(CC)]
        qt = big.tile([128, n], BF16, tag="qt", name="qt")
        ktB = big.tile([128, n], BF16, tag="ktB", name="ktB")
        v2 = big.tile([128, 130 * mb_cnt], BF16, tag="v2", name="v2")
        nc.vector.memset(v2[:], 1.0)
        atB = big.tile([128, n], BF16, tag="atB", name="atB")

        # ---- prep: per n-block compute Q^T, K^T, V (x DMAs issued upfront) ----
        for nb in range(nb_cnt):
            ns = slice(nb * 512, (nb + 1) * 512)
            for cc in range(CC):
                nc.sync.dma_start(out=xtb[cc][:, ns],
                                  in_=xT[cc * 128:(cc + 1) * 128, ns])

        def emit_prep(nb):
            ns = slice(nb * 512, (nb + 1) * 512)
            psqk = ps_T.tile([128, 1024], F32, tag="T", name="psqk")
            for blk in range(2):  # 0: Q rows (prescaled), 1: K rows
                for cc in range(CC):
                    nc.tensor.matmul(
                        psqk[:, blk * 512:(blk + 1) * 512],
                        lhsT=wq[cc][:, blk * 128:(blk + 1) * 128],
                        rhs=xtb[cc][:, ns],
                        start=(cc == 0),
                        stop=(cc == CC - 1),
                    )
            nc.scalar.copy(out=qt[:, ns], in_=psqk[:, 0:512])
            nc.scalar.copy(out=ktB[:, ns], in_=psqk[:, 512:1024])
            # V for the 4 key-chunks of this n-block: [128 keys, 128 vchan] each
            psv = ps_T.tile([128, 1024], F32, tag="T", name="psv")
            for j in range(4):
                mb = nb * 4 + j
                for cc in range(CC):
                    nc.tensor.matmul(
                        psv[:, j * 128:(j + 1) * 128],
                        lhsT=xtb[cc][:, mb * 128:(mb + 1) * 128],
                        rhs=wq[cc][:, 256:384],
                        start=(cc == 0),
                        stop=(cc == CC - 1),
                    )
            # scatter into v2 layout [1 | V_h0(64) | 1 | V_h1(64)] per chunk:
            # the leading ones column makes the PV denominator land on PSUM
            # partition 0, where partition_broadcast can read it directly
            src = psv[:, 0:512].rearrange("p (j c) -> p j c", c=128)
            dst = v2[:, nb * 4 * 130:(nb + 1) * 4 * 130].rearrange(
                "p (j c) -> p j c", c=130)
            for h in range(2):
                nc.vector.tensor_copy(
                    out=dst[:, :, h * 65 + 1:h * 65 + 65],
                    in_=src[:, :, h * 64:(h + 1) * 64],
                )

        emit_prep(0)
        if nb_cnt > 1:
            emit_prep(1)

        # ---- attention ----
        # po rows: 0 = denominator (leading ones column), 1:65 = y_u
        def emit_norms(nb, po0, po1):
            ns = slice(nb * 512, (nb + 1) * 512)
            yvs, recs = [], []
            for po in (po0, po1):
                yu = nrm.tile([65, 512], F32, tag="yu", name="yu")
                nc.vector.tensor_copy(out=yu[:], in_=po[0:65, :])
                rc1 = nrm.tile([1, 512], F32, tag="rc1", name="rc1")
                nc.vector.reciprocal_approx_fast(out=rc1[:], in_=yu[0:1, :])
                # partition shift 1:65 -> 0:64 rides the DMA engine, hidden
                # behind the reciprocal + broadcast chain
                yv = nrm.tile([64, 512], F32, tag="yv", name="yv")
                nc.sync.dma_start(out=yv[:], in_=yu[1:65, :])
                yvs.append(yv)
                recs.append(rc1)
            brs = []
            for rc1 in recs:
                rec = nrm.tile([64, 512], F32, tag="rec", name="rec")
                nc.gpsimd.partition_broadcast(rec[:], rc1[0:1, :])
                brs.append(rec)
            nc.vector.tensor_mul(out=atB[0:64, ns], in0=yvs[0][:],
                                 in1=brs[0][:])
            a1 = nrm.tile([64, 512], BF16, tag="a1", name="a1")
            nc.vector.tensor_mul(out=a1[:], in0=yvs[1][:], in1=brs[1][:])
            nc.sync.dma_start(out=atB[64:128, ns], in_=a1[:])

        def emit_proj_ob(nb, ob):
            ns = slice(nb * 512, (nb + 1) * 512)
            pp = ps_T.tile([128, 1024], F32, tag="T", name="pp")[:, 0:512]
            nc.tensor.matmul(
                pp,
                lhsT=wpb[:, ob * 128:(ob + 1) * 128],
                rhs=atB[:, ns],
                start=True,
                stop=True,
            )
            ys = ysp.tile([128, 512], BF16, tag="ys", name="ys")
            nc.scalar.copy(out=ys[:], in_=pp)
            nc.sync.dma_start(out=out[ob * 128:(ob + 1) * 128, ns], in_=ys[:])

        def emit_pv(m, es, po0, po1):
            for h, po in ((0, po0), (1, po1)):
                nc.tensor.matmul(
                    po[0:65, :],
                    lhsT=v2[:, m * 130 + 65 * h:m * 130 + 65 * h + 65],
                    rhs=es[:, h * 512:(h + 1) * 512],
                    start=(m == 0),
                    stop=(m == mb_cnt - 1),
                )

        # Chunks are processed in batches of 2 (QK pair x2, then the lagged
        # PVs) to halve the QK<->PV transitions whose LDWEIGHTS don't hide.
        # PV lags QK/exp by 4-5 chunks so exp latency stays off the critical
        # path; prep for n-blocks 1.. is interleaved into nb=0's chunk loop.
        pend = None  # (nb, po0, po1, {m: es for last 4 chunks})
        for nb in range(nb_cnt):
            ns = slice(nb * 512, (nb + 1) * 512)
            po0 = ps_O.tile([128, 512], F32, tag="po", name="po0")
            po1 = ps_O.tile([128, 512], F32, tag="po", name="po1")
            es_hist = {}
            for mm in range(0, mb_cnt, 2):
                for m in (mm, mm + 1):
                    ks = slice(m * 128, (m + 1) * 128)
                    T = ps_T.tile([128, 1024], F32, tag="T", name="T")
                    # row-tiled QK pair: head0 on rows 0-63, head1 on 64-127
                    nc.tensor.matmul(T[:, 0:512], lhsT=ktB[0:64, ks],
                                     rhs=qt[0:64, ns], start=True, stop=True)
                    nc.tensor.matmul(T[:, 512:1024], lhsT=ktB[64:128, ks],
                                     rhs=qt[64:128, ns], start=True, stop=True)
                    es = esp.tile([128, 1024], BF16, tag="es", name="es")
                    if (m % 16) in DVE_M:
                        nc.vector.tensor_scalar(
                            out=es[:].bitcast(I16), in0=T[:],
                            scalar1=BOFS, scalar2=None, op0=ADD)
                    else:
                        nc.scalar.activation(out=es[:], in_=T[:], func=EXP,
                                             scale=SCALE_SC)
                    es_hist[m] = es
                if pend is not None:
                    pnb, ppo0, ppo1, pes = pend
                    if mm == 0:
                        emit_pv(mb_cnt - 4, pes[mb_cnt - 4], ppo0, ppo1)
                        emit_pv(mb_cnt - 3, pes[mb_cnt - 3], ppo0, ppo1)
                    elif mm == 2:
                        emit_pv(mb_cnt - 2, pes[mb_cnt - 2], ppo0, ppo1)
                        emit_pv(mb_cnt - 1, pes[mb_cnt - 1], ppo0, ppo1)
                        emit_norms(pnb, ppo0, ppo1)
                        pend = None
                if nb == 0 and mm % 4 == 0 and mm <= 20 \
                        and (mm // 4 + 2) < nb_cnt:
                    emit_prep(mm // 4 + 2)
                if nb > 0 and mm in (10, 12):
                    emit_proj_ob(nb - 1, mm - 10)
                    emit_proj_ob(nb - 1, mm - 9)
                for m in (mm - 4, mm - 3):
                    if m >= 0:
                        emit_pv(m, es_hist.pop(m), po0, po1)
            pend = (nb, po0, po1,
                    {m: es_hist.pop(m) for m in range(mb_cnt - 4, mb_cnt)})

        pnb, ppo0, ppo1, pes = pend
        for m in range(mb_cnt - 4, mb_cnt):
            emit_pv(m, pes[m], ppo0, ppo1)
        emit_norms(pnb, ppo0, ppo1)
        for ob in range(4):
            emit_proj_ob(nb_cnt - 1, ob)

    nc.compile()
    return nc


_NC_CACHE = None
LAST_EXEC_NS = None


def kernel(x, w_qkv, w_proj, b_proj):
    global _NC_CACHE, LAST_EXEC_NS
    x = np.asarray(x, dtype=np.float32)
    w_qkv = np.asarray(w_qkv, dtype=np.float32)
    w_proj = np.asarray(w_proj, dtype=np.float32)
    b_proj = np.asarray(b_proj, dtype=np.float32)
    B = x.shape[0]

    if _NC_CACHE is None:
        _NC_CACHE = build_nc()
    nc = _NC_CACHE

    xTs = [np.ascontiguousarray(x[b].T.astype(BF16_NP)) for b in range(B)]
    in_maps = []
    for c in range(8):
        b, hp = c // 4, c % 4
        qr = w_qkv[2 * hp * 64:2 * hp * 64 + 128] * np.float32(CFAC)
        kr = w_qkv[512 + 2 * hp * 64:512 + 2 * hp * 64 + 128]
        vr = w_qkv[1024 + 2 * hp * 64:1024 + 2 * hp * 64 + 128]
        wqkvT = np.ascontiguousarray(
            np.concatenate([qr, kr, vr], 0).T.astype(BF16_NP))
        wpT = np.ascontiguousarray(
            w_proj[:, hp * 128:(hp + 1) * 128].T.astype(BF16_NP))
        in_maps.append({"xT": xTs[b], "wqkvT": wqkvT, "wpT": wpT})

    res = run_bass_kernel_spmd(
        nc,
        in_maps,
        core_ids=list(range(8)),
        trace=bool(int(os.environ.get("ATTN_TRACE", "0"))),
    )
    LAST_EXEC_NS = res.exec_time_ns

    out = np.zeros((B, N, DIM), np.float32)
    for b in range(B):
        acc = res.results[4 * b]["out"].astype(np.float32)
        for c in range(4 * b + 1, 4 * b + 4):
            acc += res.results[c]["out"].astype(np.float32)
        out[b] = acc.T + b_proj
    return out
